# revision 1
# baseline (speedup 1.0000x reference)
"""CRF loss (2-state FSA) on 8 Trainium2 NeuronCores.

Math: with y = exp(log_probs), the per-step denominator scores are linear in y:
  E0 = log S0, S0 = sum_c y[c]*U0[c];  E1 = log S1, S1 = sum_c y[c]*U1[c]
where U0/U1 are softmax segments of den_scores mapped through the arc table.
The 2-state forward recurrence runs in REAL space as products of 2x2 matrices
  M_t = [[S0, S1], [p*e00, p*e11]],  p = y[2] = exp(lp[:, 2])
composed on-device over chunks of L=8 steps (scaled by 32 per step to avoid
underflow; exact correction 8*ln(32) removed on host). Steps past input_len
become 32*I (identity under uniform scaling). The host folds per-sequence
chunk matrices in log space (gather/unshard-scale work) and sums partials.

Numerator: per-position gather lp[bt, label] == ln(y16[bt, label]) extracted
with tensor_mask_reduce (per-partition [label, label+1) range mask + max).

Sharding: data-parallel over batch; core k owns sequences [8k, 8k+8).
Per-core layout: partition p holds 256 consecutive (b, t) rows; sequence of a
partition = p // 16, t-offset = (p % 16) * 256 (fully contiguous DMA loads).
"""

import os
import sys

import numpy as np

for _p in ("/opt/trn_rl_repo", os.path.expanduser("~/.axon_site/_ro/trn_rl_repo")):
    if os.path.isdir(_p) and _p not in sys.path:
        sys.path.insert(0, _p)

import concourse.bacc as bacc
import concourse.bass as bass
import concourse.mybir as mybir
import concourse.tile as tile
from concourse.bass_utils import run_bass_kernel_spmd

F32 = mybir.dt.float32
BF16 = mybir.dt.bfloat16
I32 = mybir.dt.int32
Alu = mybir.AluOpType
Act = mybir.ActivationFunctionType

L = 125
C = 128          # symbol classes
B, T = 64, 4096
NCORES = 8
BSH = B // NCORES            # sequences per core = 8
BT = BSH * T                 # positions per core = 32768
NI = BT // 128               # free positions per partition = 256
NQ = 4                       # quarters (DMA/compute pipelining)
NIQ = NI // NQ               # 64 positions per quarter
LCH = 2                      # scan chunk length (steps composed on device)
NCH = NI // LCH              # 128 chunk matrices per partition
SCALE = 32.0                 # per-step scaling against fp32 underflow
NEGBIG = -3.0e38


def _build_program():
    nc = bacc.Bacc("TRN2", target_bir_lowering=False, debug=False)

    lp_d = nc.dram_tensor("lp", [BT, C], F32, kind="ExternalInput")
    lab_d = nc.dram_tensor("lab", [128, NI], I32, kind="ExternalInput")
    lens_d = nc.dram_tensor("lens", [BSH, 1], F32, kind="ExternalInput")
    den_d = nc.dram_tensor("den2", [2, C], F32, kind="ExternalInput")
    iota_d = nc.dram_tensor("iota_i", [128, NI], F32, kind="ExternalInput")
    iotac_d = nc.dram_tensor("iota_c", [128, C], F32, kind="ExternalInput")
    offs_d = nc.dram_tensor("offs", [128, 1], F32, kind="ExternalInput")
    e8_d = nc.dram_tensor("e8", [BSH, 128], F32, kind="ExternalInput")
    i2_d = nc.dram_tensor("i2", [2, 2], F32, kind="ExternalInput")
    sel_d = nc.dram_tensor("sel01", [2, 2, 128], F32, kind="ExternalInput")

    chunklog_d = nc.dram_tensor("chunklog", [128, 4, NCH], F32, kind="ExternalOutput")
    numpart_d = nc.dram_tensor("numpart", [128, 1], F32, kind="ExternalOutput")
    fs_d = nc.dram_tensor("fs", [1, 1], F32, kind="ExternalOutput")

    with tile.TileContext(nc) as tc:
        with (
            tc.tile_pool(name="const", bufs=1) as cpool,
            tc.tile_pool(name="small", bufs=1) as spool,
            tc.tile_pool(name="lp16", bufs=2) as lp_pool,
            tc.tile_pool(name="y16", bufs=2) as y_pool,
            tc.tile_pool(name="yT", bufs=1) as yT_pool,
            tc.tile_pool(name="tmr", bufs=2) as tmr_pool,
            tc.tile_pool(name="scan", bufs=1) as scan_pool,
            tc.tile_pool(name="psS", bufs=4, space=bass.MemorySpace.PSUM) as psS,
            tc.tile_pool(name="psU", bufs=1, space=bass.MemorySpace.PSUM) as psU,
        ):
            # ---------------- constants ----------------
            iota_i = cpool.tile([128, NI], F32)
            nc.sync.dma_start(iota_i[:], iota_d.ap())
            iota_c = cpool.tile([128, C], F32)
            nc.sync.dma_start(iota_c[:], iotac_d.ap())
            offs = cpool.tile([128, 1], F32)
            nc.sync.dma_start(offs[:], offs_d.ap())
            e8 = cpool.tile([BSH, 128], F32)
            nc.sync.dma_start(e8[:], e8_d.ap())
            i2 = cpool.tile([2, 2], F32)
            nc.sync.dma_start(i2[:], i2_d.ap())
            sel0 = cpool.tile([2, 128], F32)
            nc.sync.dma_start(sel0[:], sel_d.ap()[0:1, :, :].rearrange("a k m -> (a k) m"))
            sel1 = cpool.tile([2, 128], F32)
            nc.sync.dma_start(sel1[:], sel_d.ap()[1:2, :, :].rearrange("a k m -> (a k) m"))
            den_sb = cpool.tile([2, C], F32)
            nc.sync.dma_start(den_sb[:], den_d.ap())
            lens_sb = cpool.tile([BSH, 1], F32)
            nc.sync.dma_start(lens_sb[:], lens_d.ap())
            lab_sb = cpool.tile([128, NI], I32)
            nc.sync.dma_start(lab_sb[:], lab_d.ap())

            # ---------------- arc weights (u = per-state softmax) ----------------
            ed = spool.tile([2, C], F32)
            nc.scalar.activation(ed[:], den_sb[:], Act.Exp)
            zd = spool.tile([2, 1], F32)
            nc.vector.tensor_reduce(zd[:], ed[:], mybir.AxisListType.X, Alu.add)
            rd = spool.tile([2, 1], F32)
            nc.vector.reciprocal(rd[:], zd[:])
            u = spool.tile([2, C], F32)
            nc.vector.tensor_scalar(u[:], ed[:], rd[:, 0:1], None, Alu.mult)

            # urows[0, c] = E0 weight of class c, urows[1, c] = E1 weight
            urows = spool.tile([2, C], F32)
            nc.gpsimd.memset(urows[:], 0.0)
            nc.vector.tensor_copy(urows[0:1, 1:2], u[0:1, 0:1])
            nc.vector.tensor_copy(urows[0:2, 3:128], u[0:2, 1:126])
            u_ps = psU.tile([128, 2], F32)
            nc.tensor.matmul(u_ps[:], urows[:], i2[:], start=True, stop=True)
            # split-bf16 weights: U = hi + lo keeps effective fp32 precision
            # through the bf16 matmul (two PSUM-accumulated matmuls per block)
            uf = spool.tile([128, 2], F32)
            nc.vector.tensor_copy(uf[:], u_ps[:])
            u16 = spool.tile([128, 4], BF16)
            nc.vector.tensor_copy(u16[:, 0:2], uf[:])
            ulo = spool.tile([128, 2], F32)
            nc.vector.tensor_sub(ulo[:], uf[:], u16[:, 0:2])
            nc.vector.tensor_copy(u16[:, 2:4], ulo[:])

            # e_b[:, 0] = e00 = u0[126] bcast, e_b[:, 1] = e11 = u1[0] bcast
            # (selector matmuls: sel0/sel1 pick row 0 / row 1 of u)
            e_ps = psU.tile([128, 2], F32)
            nc.tensor.matmul(
                e_ps[:, 0:1], sel0[:], u[0:2, 126:127], start=True, stop=True
            )
            nc.tensor.matmul(
                e_ps[:, 1:2], sel1[:], u[0:2, 0:1], start=True, stop=True
            )
            e_b = spool.tile([128, 2], F32)
            nc.vector.tensor_copy(e_b[:], e_ps[:])

            # final arc score (log u0[127]) -> output
            fs_sb = spool.tile([1, 1], F32)
            nc.scalar.activation(fs_sb[:], u[0:1, 127:128], Act.Ln)
            nc.sync.dma_start(fs_d.ap(), fs_sb[:])

            # ---------------- per-partition length mask ----------------
            thr_ps = psU.tile([128, 1], F32)
            nc.tensor.matmul(thr_ps[:], e8[:], lens_sb[:], start=True, stop=True)
            thr = spool.tile([128, 1], F32)
            nc.vector.tensor_tensor(thr[:], thr_ps[:], offs[:], Alu.subtract)
            m01 = spool.tile([128, NI], F32)
            nc.vector.tensor_scalar(m01[:], iota_i[:], thr[:, 0:1], None, Alu.is_lt)
            w32 = spool.tile([128, NI], F32)   # 32*m
            nc.vector.tensor_scalar(w32[:], m01[:], SCALE, None, Alu.mult)
            w32c = spool.tile([128, NI], F32)  # 32*(1-m)
            nc.vector.tensor_scalar(w32c[:], m01[:], -SCALE, SCALE, Alu.mult, Alu.add)

            # labels with validity folded in: invalid positions -> 200 (no
            # class matches, so masked positions contribute exactly 0)
            labf = spool.tile([128, NI], F32)
            nc.vector.tensor_copy(labf[:], lab_sb[:])
            nc.vector.tensor_scalar(labf[:], labf[:], -200.0, None, Alu.add)
            nc.vector.tensor_tensor(labf[:], labf[:], m01[:], Alu.mult)
            nc.vector.tensor_scalar(labf[:], labf[:], 200.0, None, Alu.add)

            # ---------------- persistent main buffers ----------------
            s01 = spool.tile([128, 2 * NI], F32)    # S0/S1 interleaved by position
            p_buf = spool.tile([128, NI], F32)      # y[:, 2] per position
            numq = spool.tile([128, 16], F32)        # per-group numerator sums
            yT = yT_pool.tile([128, BT], BF16)      # transposed exp, block-major

            lp_view = lp_d.ap().rearrange("(p i) c -> p i c", p=128)

            # ---------------- streaming main loop ----------------
            for q in range(NQ):
                lp32 = lp_pool.tile([128, NIQ * C], F32)
                nc.sync.dma_start(
                    lp32[:], lp_view[:, q * NIQ : (q + 1) * NIQ, :]
                )
                y16 = y_pool.tile([128, NIQ * C], BF16)
                nc.scalar.activation(y16[:], lp32[:], Act.Exp)

                y3 = y16[:].rearrange("p (i c) -> p i c", c=C)
                nc.scalar.copy(
                    p_buf[:, q * NIQ : (q + 1) * NIQ], y3[:, :, 2:3]
                )

                yT_q = yT[:, q * NIQ * C : (q + 1) * NIQ * C].rearrange(
                    "p (j z) -> p j z", z=128
                )
                nc.sync.dma_start_transpose(yT_q, y16[:])

                for g in range(NIQ // 8):
                    ps = psS.tile([128, 16], F32)
                    for jj in range(8):
                        j = g * 8 + jj
                        blk = yT[
                            :, (q * NIQ + j) * 128 : (q * NIQ + j + 1) * 128
                        ]
                        nc.tensor.matmul(
                            ps[:, 2 * jj : 2 * jj + 2], blk, u16[:, 0:2],
                            start=True, stop=False,
                        )
                        nc.tensor.matmul(
                            ps[:, 2 * jj : 2 * jj + 2], blk, u16[:, 2:4],
                            start=False, stop=True,
                        )
                    i0 = q * NIQ + g * 8
                    nc.scalar.copy(s01[:, 2 * i0 : 2 * i0 + 16], ps[:])

                # numerator gather: (iota_c == label) * lp, one nonzero term
                # per position (exact fp32). DVE accum_out crashes the device
                # on this stack, so the group sums run on the scalar engine
                # (ACT accum_out — HW-verified safe); invalid positions were
                # already zeroed via the label-folding above.
                GRP = 16
                for h in range(NIQ // GRP):
                    mout = tmr_pool.tile([128, GRP * C], F32)
                    for j in range(GRP):
                        jq = h * GRP + j
                        i = q * NIQ + jq
                        nc.vector.scalar_tensor_tensor(
                            mout[:, j * C : (j + 1) * C],
                            iota_c[:],
                            labf[:, i : i + 1],
                            lp32[:, jq * C : (jq + 1) * C],
                            Alu.is_equal,
                            Alu.mult,
                        )
                    scr = tmr_pool.tile([128, GRP * C], BF16, tag="scr")
                    nc.scalar.activation(
                        scr[:], mout[:], Act.Identity,
                        accum_out=numq[:, 4 * q + h : 4 * q + h + 1],
                    )

            # ---------------- numerator ----------------
            numpart = spool.tile([128, 1], F32)
            nc.vector.tensor_reduce(
                numpart[:], numq[:], mybir.AxisListType.X, Alu.add
            )
            nc.sync.dma_start(numpart_d.ap(), numpart[:])

            # ---------------- scan step tensors (masked + scaled) ----------------
            s0v = s01[:].rearrange("p (i two) -> p i two", two=2)[:, :, 0:1]
            s1v = s01[:].rearrange("p (i two) -> p i two", two=2)[:, :, 1:2]
            sm0 = spool.tile([128, NI], F32)
            nc.gpsimd.tensor_tensor(sm0[:], s0v, w32[:], Alu.mult)
            nc.gpsimd.tensor_tensor(sm0[:], sm0[:], w32c[:], Alu.add)
            sm1 = spool.tile([128, NI], F32)
            nc.gpsimd.tensor_tensor(sm1[:], s1v, w32[:], Alu.mult)
            a_t = spool.tile([128, NI], F32)
            nc.gpsimd.tensor_scalar(a_t[:], p_buf[:], e_b[:, 0:1], None, Alu.mult)
            nc.gpsimd.tensor_tensor(a_t[:], a_t[:], w32[:], Alu.mult)
            b_t = spool.tile([128, NI], F32)
            nc.gpsimd.tensor_scalar(b_t[:], p_buf[:], e_b[:, 1:2], None, Alu.mult)
            nc.gpsimd.tensor_tensor(b_t[:], b_t[:], w32[:], Alu.mult)
            nc.gpsimd.tensor_tensor(b_t[:], b_t[:], w32c[:], Alu.add)

            def step_slice(tile_ap, t):
                return tile_ap[:].rearrange("p (c l) -> p c l", l=LCH)[:, :, t : t + 1]

            # ---------------- on-device chunk scan (real space) ----------------
            P = {}
            for name, src in (("00", sm0), ("01", sm1), ("10", a_t), ("11", b_t)):
                pt = scan_pool.tile([128, NCH], F32, tag=f"P{name}")
                nc.scalar.copy(pt[:], step_slice(src, 0))
                P[name] = pt

            for t in range(1, LCH):
                s0t, s1t = step_slice(sm0, t), step_slice(sm1, t)
                att, btt = step_slice(a_t, t), step_slice(b_t, t)
                newP = {}
                for col in ("0", "1"):
                    pc0, pc1 = P["0" + col], P["1" + col]
                    n0 = scan_pool.tile([128, NCH], F32, tag=f"n0{col}")
                    t1 = scan_pool.tile([128, NCH], F32, tag="t1")
                    nc.gpsimd.tensor_tensor(t1[:], s0t, pc0[:], Alu.mult)
                    nc.gpsimd.tensor_tensor(n0[:], s1t, pc1[:], Alu.mult)
                    nc.gpsimd.tensor_tensor(n0[:], t1[:], n0[:], Alu.add)
                    n1 = scan_pool.tile([128, NCH], F32, tag=f"n1{col}")
                    t2 = scan_pool.tile([128, NCH], F32, tag="t2")
                    nc.gpsimd.tensor_tensor(t2[:], att, pc0[:], Alu.mult)
                    nc.gpsimd.tensor_tensor(n1[:], btt, pc1[:], Alu.mult)
                    nc.gpsimd.tensor_tensor(n1[:], t2[:], n1[:], Alu.add)
                    newP["0" + col] = n0
                    newP["1" + col] = n1
                P = newP

            for e, name in enumerate(("00", "01", "10", "11")):
                plog = scan_pool.tile([128, NCH], F32, tag="plog")
                nc.scalar.activation(plog[:], P[name][:], Act.Ln)
                nc.sync.dma_start(chunklog_d.ap()[:, e : e + 1, :], plog[:])

    nc.compile()
    return nc


_NC_CACHE = None


def _get_program():
    global _NC_CACHE
    if _NC_CACHE is None:
        _NC_CACHE = _build_program()
    return _NC_CACHE


def _make_in_maps(log_probs, den_scores, input_lens, labels):
    pids = np.arange(128)
    iota_i = np.broadcast_to(
        np.arange(NI, dtype=np.float32), (128, NI)
    ).copy()
    iota_c = np.broadcast_to(
        np.arange(C, dtype=np.float32), (128, C)
    ).copy()
    offs = ((pids % 16) * NI).astype(np.float32).reshape(128, 1)
    e8 = (pids[None, :] // 16 == np.arange(BSH)[:, None]).astype(np.float32)
    i2 = np.eye(2, dtype=np.float32)
    sel01 = np.zeros((2, 2, 128), dtype=np.float32)
    sel01[0, 0, :] = 1.0
    sel01[1, 1, :] = 1.0
    den2 = np.full((2, C), -1e30, dtype=np.float32)
    den2[0, :] = den_scores[: L + 3]
    den2[1, : L + 1] = den_scores[L + 3 :]

    in_maps = []
    for k in range(NCORES):
        sl = slice(k * BSH, (k + 1) * BSH)
        in_maps.append(
            dict(
                lp=np.ascontiguousarray(
                    log_probs[sl].reshape(BT, C), dtype=np.float32
                ),
                lab=np.ascontiguousarray(
                    labels[sl].reshape(128, NI), dtype=np.int32
                ),
                lens=input_lens[sl].astype(np.float32).reshape(BSH, 1),
                den2=den2,
                iota_i=iota_i,
                iota_c=iota_c,
                offs=offs,
                e8=e8,
                i2=i2,
                sel01=sel01,
            )
        )
    return in_maps


def _combine_host(results):
    """Fold per-core device outputs into the scalar loss (float64 host fold)."""
    num = 0.0
    logM_all = []  # [64, NCHUNKS_TOTAL, 2, 2] in global sequence order
    fs = None
    corr = LCH * np.log(SCALE)
    for res in results:
        num += float(res["numpart"].sum(dtype=np.float64))
        fs = float(res["fs"][0, 0])
        cl = res["chunklog"].astype(np.float64)  # [128, 4, NCH]
        # partition p -> (seq_local = p//16, toff = p%16); chunk order (toff, c)
        cl = cl.reshape(BSH, 16, 4, NCH)
        cl = np.transpose(cl, (0, 1, 3, 2)).reshape(BSH, 16 * NCH, 2, 2)
        logM_all.append(cl - corr)
    mats = np.concatenate(logM_all, axis=0)  # [64, 512, 2, 2]

    def compose(Bm, Am):
        # C = B o A : C[i,j] = LSE_k(B[i,k] + A[k,j])
        s = Bm[..., :, :, None] + Am[..., None, :, :]  # [..., i, k, j]
        return _lse(s, axis=-2)

    while mats.shape[1] > 1:
        n = mats.shape[1]
        if n % 2:
            last = mats[:, -1:]
            mats = compose(mats[:, 1::2], mats[:, 0:-1:2])
            mats = np.concatenate([mats, last], axis=1)
        else:
            mats = compose(mats[:, 1::2], mats[:, 0::2])
    den = float(mats[:, 0, 0, 0].sum()) + B * fs
    return np.float32(num - den)


def _lse(x, axis):
    m = np.max(x, axis=axis, keepdims=True)
    m = np.where(np.isfinite(m), m, 0.0)
    out = np.squeeze(m, axis) + np.log(
        np.sum(np.exp(x - m), axis=axis)
    )
    return out


def kernel(log_probs, den_scores, input_lens, labels):
    nc = _get_program()
    in_maps = _make_in_maps(
        np.asarray(log_probs), np.asarray(den_scores),
        np.asarray(input_lens), np.asarray(labels),
    )
    res = run_bass_kernel_spmd(nc, in_maps, core_ids=list(range(NCORES)))
    return _combine_host(res.results)



# revision 2
# speedup vs baseline: 1.8840x; 1.8840x over previous
"""CRF loss (2-state FSA) on 8 Trainium2 NeuronCores — v2.

Math: with y = exp(log_probs), the per-step denominator scores are linear in
y, so the 2-state forward recurrence runs in REAL space as products of 2x2
matrices M_t = [[S0, S1], [e00*p, e11*p]] (p = y[:, 2]), composed on-device
over chunks of LCH=2 steps, scaled by 32/step against fp32 underflow (exact
correction removed in the host fold). Steps past input_len become 32*I.

Device layout (per core, 8 sequences / 32768 positions): the host ships
log-probs TRANSPOSED — classes on the 128 partitions, positions along the
free axis in (j, q) order so that position q*256+j lands in column j*128+q.
PE matmuls over each 128-column block then produce (32*S0, 32*S1, 32*e00*p,
32*e11*p) per position directly in the scan layout: PSUM partition q holds
the 256 consecutive positions of partition q, block index j as the free
axis. No on-device transpose and no DMA-transpose traffic.

Shipping lpT in fp16 halves HBM traffic (the memory bottleneck); the arc
weights ride in the matmul rhs as bf16 hi+lo splits for fp32-equivalent
weight precision.

Numerator: the host re-encodes labels as a one-hot matrix in the same
transposed layout (fp8, 0/1 exact, invalid positions zeroed). The PE
accumulates D += lpT_j^T @ onehot_j over all 256 blocks in one PSUM tile;
diag(D)[q] = sum_j lp[q*256+j, lab] so trace(D) is the core's numerator.

Host: softmax of the 254 den_scores (constant prep), the log-space fold of
the per-sequence chunk matrices, and the final num - den reduction.
"""

import os
import sys

import ml_dtypes
import numpy as np

for _p in ("/opt/trn_rl_repo", os.path.expanduser("~/.axon_site/_ro/trn_rl_repo")):
    if os.path.isdir(_p) and _p not in sys.path:
        sys.path.insert(0, _p)

import concourse.bacc as bacc
import concourse.bass as bass
import concourse.mybir as mybir
import concourse.tile as tile
from concourse.bass_utils import run_bass_kernel_spmd

F32 = mybir.dt.float32
F16 = mybir.dt.float16
BF16 = mybir.dt.bfloat16
FP8 = mybir.dt.float8e4
Alu = mybir.AluOpType
Act = mybir.ActivationFunctionType

L = 125
C = 128          # symbol classes
B, T = 64, 4096
NCORES = 8
BSH = B // NCORES            # sequences per core = 8
BT = BSH * T                 # positions per core = 32768
NI = BT // 128               # positions per partition = 256
NQ = 4                       # quarters (DMA/compute pipelining)
NIQ = NI // NQ               # 64 blocks per quarter
NBLK = BT // 128             # 256 column blocks
LCH = 2                      # scan chunk length
NCH = NI // LCH              # 128 chunk matrices per partition
SCALE = 32.0                 # per-step scaling against fp32 underflow


def _build_program():
    nc = bacc.Bacc("TRN2", target_bir_lowering=False, debug=False)

    lp_d = nc.dram_tensor("lpt", [128, BT], F16, kind="ExternalInput")
    oh_d = nc.dram_tensor("oh", [128, BT], FP8, kind="ExternalInput")
    u_d = nc.dram_tensor("u16", [128, 8], BF16, kind="ExternalInput")
    mm_d = nc.dram_tensor("mm", [128, NI], F32, kind="ExternalInput")
    wc_d = nc.dram_tensor("wc", [128, NI], F32, kind="ExternalInput")

    chunklog_d = nc.dram_tensor("chunklog", [128, 4, NCH], F32, kind="ExternalOutput")
    dmat_d = nc.dram_tensor("dmat", [128, 128], F32, kind="ExternalOutput")

    with tile.TileContext(nc) as tc:
        with (
            tc.tile_pool(name="const", bufs=1) as cpool,
            tc.tile_pool(name="lp", bufs=2) as lp_pool,
            tc.tile_pool(name="oh", bufs=2) as oh_pool,
            tc.tile_pool(name="y", bufs=2) as y_pool,
            tc.tile_pool(name="scan", bufs=1) as spool,
            tc.tile_pool(name="psS", bufs=1, space=bass.MemorySpace.PSUM) as psS,
            tc.tile_pool(name="psD", bufs=1, space=bass.MemorySpace.PSUM) as psD,
        ):
            u16 = cpool.tile([128, 8], BF16)
            nc.sync.dma_start(u16[:], u_d.ap())
            mmf = cpool.tile([128, NI], F32)
            nc.sync.dma_start(mmf[:], mm_d.ap())
            w32c = cpool.tile([128, NI], F32)
            nc.sync.dma_start(w32c[:], wc_d.ap())

            # all S-matrix entries for the whole core: [q, (j, 4)]
            sps = psS.tile([128, 4 * NI], F32)
            # numerator trace accumulator
            dps = psD.tile([128, 128], F32)

            for q in range(NQ):
                lp16 = lp_pool.tile([128, NIQ * 128], F16)
                nc.sync.dma_start(
                    lp16[:], lp_d.ap()[:, q * NIQ * 128 : (q + 1) * NIQ * 128]
                )
                oh8 = oh_pool.tile([128, NIQ * 128], FP8)
                nc.sync.dma_start(
                    oh8[:], oh_d.ap()[:, q * NIQ * 128 : (q + 1) * NIQ * 128]
                )
                y16 = y_pool.tile([128, NIQ * 128], BF16)
                nc.scalar.activation(y16[:], lp16[:], Act.Exp)

                for j in range(NIQ):
                    J = q * NIQ + j
                    yb = y16[:, j * 128 : (j + 1) * 128]
                    nc.tensor.matmul(
                        sps[:, 4 * J : 4 * J + 4], yb, u16[:, 0:4],
                        start=True, stop=False,
                    )
                    nc.tensor.matmul(
                        sps[:, 4 * J : 4 * J + 4], yb, u16[:, 4:8],
                        start=False, stop=True,
                    )
                    nc.tensor.matmul(
                        dps[:],
                        lp16[:, j * 128 : (j + 1) * 128],
                        oh8[:, j * 128 : (j + 1) * 128],
                        start=(J == 0), stop=(J == NBLK - 1),
                        skip_group_check=True,
                    )

            # ---------------- numerator out ----------------
            dsb = spool.tile([128, 128], F32)
            nc.vector.tensor_copy(dsb[:], dps[:])
            nc.sync.dma_start(dmat_d.ap(), dsb[:])

            # ---------------- scan prep (DVE reads strided PSUM) ----------------
            s4 = sps[:].rearrange("p (j four) -> p j four", four=4)
            sm0 = spool.tile([128, NI], F32)
            nc.vector.tensor_tensor(sm0[:], s4[:, :, 0:1], mmf[:], Alu.mult)
            nc.vector.tensor_tensor(sm0[:], sm0[:], w32c[:], Alu.add)
            sm1 = spool.tile([128, NI], F32)
            nc.vector.tensor_tensor(sm1[:], s4[:, :, 1:2], mmf[:], Alu.mult)
            a_t = spool.tile([128, NI], F32)
            nc.vector.tensor_tensor(a_t[:], s4[:, :, 2:3], mmf[:], Alu.mult)
            b_t = spool.tile([128, NI], F32)
            nc.vector.tensor_tensor(b_t[:], s4[:, :, 3:4], mmf[:], Alu.mult)
            nc.vector.tensor_tensor(b_t[:], b_t[:], w32c[:], Alu.add)

            def step_slice(tile_ap, t):
                return tile_ap[:].rearrange("p (c l) -> p c l", l=LCH)[:, :, t : t + 1]

            # ---------------- on-device chunk scan (real space) ----------------
            P = {}
            for name, src in (("00", sm0), ("01", sm1), ("10", a_t), ("11", b_t)):
                pt = spool.tile([128, NCH], F32, tag=f"P{name}")
                nc.gpsimd.tensor_copy(pt[:], step_slice(src, 0))
                P[name] = pt

            for t in range(1, LCH):
                s0t, s1t = step_slice(sm0, t), step_slice(sm1, t)
                att, btt = step_slice(a_t, t), step_slice(b_t, t)
                newP = {}
                for col in ("0", "1"):
                    pc0, pc1 = P["0" + col], P["1" + col]
                    n0 = spool.tile([128, NCH], F32, tag=f"n0{col}")
                    t1 = spool.tile([128, NCH], F32, tag="t1")
                    nc.gpsimd.tensor_tensor(t1[:], s0t, pc0[:], Alu.mult)
                    nc.gpsimd.tensor_tensor(n0[:], s1t, pc1[:], Alu.mult)
                    nc.gpsimd.tensor_tensor(n0[:], t1[:], n0[:], Alu.add)
                    n1 = spool.tile([128, NCH], F32, tag=f"n1{col}")
                    t2 = spool.tile([128, NCH], F32, tag="t2")
                    nc.gpsimd.tensor_tensor(t2[:], att, pc0[:], Alu.mult)
                    nc.gpsimd.tensor_tensor(n1[:], btt, pc1[:], Alu.mult)
                    nc.gpsimd.tensor_tensor(n1[:], t2[:], n1[:], Alu.add)
                    newP["0" + col] = n0
                    newP["1" + col] = n1
                P = newP

            for e, name in enumerate(("00", "01", "10", "11")):
                plog = spool.tile([128, NCH], F32, tag="plog")
                nc.scalar.activation(plog[:], P[name][:], Act.Ln)
                nc.sync.dma_start(chunklog_d.ap()[:, e : e + 1, :], plog[:])

    nc.compile()
    return nc


_NC_CACHE = None


def _get_program():
    global _NC_CACHE
    if _NC_CACHE is None:
        _NC_CACHE = _build_program()
    return _NC_CACHE


def _softmax(x):
    x = x.astype(np.float64)
    m = x.max()
    e = np.exp(x - m)
    return (e / e.sum()).astype(np.float64)


def _make_in_maps(log_probs, den_scores, input_lens, labels):
    # arc weights: per-state softmax mapped to class columns, pre-scaled by 32
    u0 = _softmax(den_scores[: L + 3])          # [128] state-0 arcs (incl final)
    u1 = _softmax(den_scores[L + 3 :])          # [126] state-1 arcs
    U = np.zeros((128, 4), np.float64)
    U[1, 0] = u0[0]                              # 'O' from state 0
    U[3:128, 0] = u0[1 : L + 1]                  # labels from state 0
    U[3:128, 1] = u1[1 : L + 1]                  # labels from state 1
    U[2, 2] = u0[L + 1]                          # e00: 0 -> 1 emitting I-
    U[2, 3] = u1[0]                              # e11: 1 -> 1 emitting I-
    U *= SCALE
    Uhi = U.astype(ml_dtypes.bfloat16)
    Ulo = (U - Uhi.astype(np.float64)).astype(ml_dtypes.bfloat16)
    u16 = np.concatenate([Uhi, Ulo], axis=1)     # [128, 8]

    # per-partition length masks: partition q holds positions [256q, 256q+256)
    pids = np.arange(128)
    seq_of_p = pids // 16
    off_of_p = (pids % 16) * NI
    thr = input_lens.reshape(NCORES, BSH)        # [core, seq]
    iota = np.arange(NI)

    lp16 = log_probs.astype(np.float16)          # [B, T, C]
    one_f8 = np.array(1.0, dtype=ml_dtypes.float8_e4m3)

    in_maps = []
    for k in range(NCORES):
        # transposed, block-permuted log probs: [c, j*128 + q] = lp[q*256+j, c]
        A = lp16[k * BSH : (k + 1) * BSH].reshape(128, NI, C)     # [q, j, c]
        lpt = np.ascontiguousarray(A.transpose(2, 1, 0)).reshape(128, BT)

        labQ = labels[k * BSH : (k + 1) * BSH].reshape(128, NI)   # [q, j]
        thr_k = (thr[k][seq_of_p] - off_of_p)                     # [128]
        validQ = iota[None, :] < thr_k[:, None]                   # [q, j]
        lab_m = np.where(validQ, labQ, -1)
        ohb = (lab_m.T[None, :, :] == np.arange(128)[:, None, None])  # [c, j, q]
        oh8 = (ohb.astype(np.uint8) * one_f8.view(np.uint8)).reshape(128, BT)
        oh8 = oh8.view(ml_dtypes.float8_e4m3)

        mmf = validQ.astype(np.float32)                           # [128, NI]
        wc = (SCALE * (1.0 - mmf)).astype(np.float32)

        in_maps.append(
            dict(lpt=lpt, oh=oh8, u16=u16.astype(ml_dtypes.bfloat16),
                 mm=mmf, wc=wc)
        )
    return in_maps


def _combine_host(results, den_scores):
    """Fold per-core device outputs into the scalar loss (float64 host fold)."""
    s0 = den_scores.astype(np.float64)[: L + 3]
    fs = float(s0[L + 2] - np.log(np.exp(s0 - s0.max()).sum()) - s0.max())
    num = 0.0
    logM_all = []
    corr = LCH * np.log(SCALE)
    for res in results:
        num += float(np.trace(res["dmat"].astype(np.float64)))
        cl = res["chunklog"].astype(np.float64)   # [128, 4, NCH]
        # partition q -> (seq_local = q//16, toff = q%16); chunk order (toff, c)
        cl = cl.reshape(BSH, 16, 4, NCH)
        cl = np.transpose(cl, (0, 1, 3, 2)).reshape(BSH, 16 * NCH, 2, 2)
        logM_all.append(cl - corr)
    mats = np.concatenate(logM_all, axis=0)       # [64, 512, 2, 2]

    def compose(Bm, Am):
        s = Bm[..., :, :, None] + Am[..., None, :, :]
        return _lse(s, axis=-2)

    while mats.shape[1] > 1:
        n = mats.shape[1]
        if n % 2:
            last = mats[:, -1:]
            mats = compose(mats[:, 1::2], mats[:, 0:-1:2])
            mats = np.concatenate([mats, last], axis=1)
        else:
            mats = compose(mats[:, 1::2], mats[:, 0::2])
    den = float(mats[:, 0, 0, 0].sum()) + B * fs
    return np.float32(num - den)


def _lse(x, axis):
    m = np.max(x, axis=axis, keepdims=True)
    m = np.where(np.isfinite(m), m, 0.0)
    out = np.squeeze(m, axis) + np.log(np.sum(np.exp(x - m), axis=axis))
    return out


def kernel(log_probs, den_scores, input_lens, labels):
    nc = _get_program()
    log_probs = np.asarray(log_probs)
    den_scores = np.asarray(den_scores)
    in_maps = _make_in_maps(
        log_probs, den_scores,
        np.asarray(input_lens), np.asarray(labels),
    )
    res = run_bass_kernel_spmd(nc, in_maps, core_ids=list(range(NCORES)))
    return _combine_host(res.results, den_scores)
